# revision 10
# baseline (speedup 1.0000x reference)
"""MultiHeadAttention TRN2 kernel: B=2, L=2048, DIM=1024, 16 heads x 64.

Sharding: 8 cores = 2 (batch) x 4 (head groups of 4 heads), tensor-parallel
on heads (Wq/Wk/Wv column-split) with Wo ROW-split: each core computes a
full-width partial output out_partial[L, DIM] from its 4 heads; the host
sums the 4 partials per batch during unshard (the all-reduce of the
row-split Wo, performed at gather time).  No device collectives.

Per core (all matmul operands fp16, PSUM accumulation fp32):
  - xT16 = q[b].T [1024, 2048], wq/wk/wv = W.T[:, headslice] [1024, 256],
    wo = Wo.T[headslice, :] [256, 1024] -- all fp16, host-converted.
  - K projected first (transposed layout KT [d, j]), then Q chunk-0, so
    scores/exp for chunk 0 start ~20us in; V (natural [j, h, 64+1] with a
    ones column -> softmax denominator) and remaining Q quarters follow.
  - scores per (chunk, head): j-tile pairs share one PSUM tile so one
    ACTIVATE exps 1024 elems; exp(0.125*s) -> fp16 attn tiles [j, i].
  - AV in NATURAL orientation (lhsT=attnT, rhs=[v|1]): out [i-block, 65]
    -- full 128-row stationary vs 65 in the transposed form (2x fewer PE
    rows), denominator lands in column 64 per i-PARTITION, so the
    normalize is a per-partition tensor_scalar (no PE broadcast matmul).
  - head pairs packed side by side [i, 128], DMA-TRANSPOSED (xbar, off-PE)
    into the Wo lhsT layout [2x64 d, i].
  - Wo partial: out[i, 1024] = sum over 2 d-pair k-tiles; PSUM -> SBUF f32
    copy on gpsimd (Pool), DMA out.  Host sums group partials.
Emission interleaves scores(c+1, h) ahead of AV(c, h) per head so the PE
stays busy while ACT (the #2 engine, ~128us of exp) catches up.
"""

import sys
from contextlib import ExitStack

import numpy as np

for _p in ("/opt/trn_rl_repo",):
    if _p not in sys.path:
        sys.path.insert(0, _p)

import concourse.bass as bass
import concourse.tile as tile
from concourse import bacc, mybir
from concourse.bass_utils import run_bass_kernel_spmd

F32 = mybir.dt.float32
F16 = mybir.dt.float16

B, L, DIM = 2, 2048, 1024
NH, HD = 16, 64           # total heads, head dim
HL = 4                    # heads per core
DL = HL * HD              # local head dims = 256
KT = DIM // 128           # 8  contraction k-tiles
JT = L // 128             # 16 j (key) tiles
CH = 512                  # i-chunk size
NCH = L // CH             # 4 chunks
NQ = 4                    # L quarters for projection streaming
QLF = L // NQ             # 512


def build_nc():
    nc = bacc.Bacc("TRN2", target_bir_lowering=False, debug=False, num_devices=8)

    xT_d = nc.dram_tensor("xT", [DIM, L], F16, kind="ExternalInput")
    wq_d = nc.dram_tensor("wq", [DIM, DL], F16, kind="ExternalInput")
    wk_d = nc.dram_tensor("wk", [DIM, DL], F16, kind="ExternalInput")
    wv_d = nc.dram_tensor("wv", [DIM, DL], F16, kind="ExternalInput")
    wo_d = nc.dram_tensor("wo", [DL, DIM], F16, kind="ExternalInput")
    out_d = nc.dram_tensor("out", [L, DIM], F32, kind="ExternalOutput")

    with tile.TileContext(nc) as tc:
        with ExitStack() as ctx:
            wpool = ctx.enter_context(tc.tile_pool(name="weights", bufs=3))
            wopool = ctx.enter_context(tc.tile_pool(name="wo", bufs=2))
            xpool = ctx.enter_context(tc.tile_pool(name="xT", bufs=33))
            qkpool = ctx.enter_context(tc.tile_pool(name="qk", bufs=16))
            vpool = ctx.enter_context(tc.tile_pool(name="v", bufs=16))
            atpool = ctx.enter_context(tc.tile_pool(name="attnT", bufs=44))
            aopool = ctx.enter_context(tc.tile_pool(name="ao", bufs=6))
            aotpool = ctx.enter_context(tc.tile_pool(name="aot", bufs=6))
            small = ctx.enter_context(tc.tile_pool(name="small", bufs=6))
            outpool = ctx.enter_context(tc.tile_pool(name="outsb", bufs=4))
            ps_proj = ctx.enter_context(
                tc.tile_pool(name="ps_proj", bufs=2, space="PSUM"))
            ps_s = ctx.enter_context(
                tc.tile_pool(name="ps_s", bufs=2, space="PSUM"))
            ps_av = ctx.enter_context(
                tc.tile_pool(name="ps_av", bufs=2, space="PSUM"))

            # ---- weights (as [128, KT, DL] k-tile stacks) ----
            def load_w(dram_t, name):
                t = wpool.tile([128, KT, DL], F16, name=name, tag="w")
                nc.sync.dma_start(
                    out=t[:], in_=dram_t[:].rearrange("(k p) n -> p k n", p=128))
                return t

            wk_sb = load_w(wk_d, "wk_sb")

            # xT loads: quarter 0 first (K proj of q0 unblocks earliest)
            xT_view = xT_d[:].rearrange("(k p) n -> k p n", p=128)
            xT_k = [[None] * KT for _ in range(NQ)]
            for qi in range(NQ):
                for k in range(KT):
                    xt = xpool.tile([128, QLF], F16, name="xt", tag="xt")
                    nc.sync.dma_start(
                        out=xt[:], in_=xT_view[k][:, qi * QLF:(qi + 1) * QLF])
                    xT_k[qi][k] = xt

            wq_sb = load_w(wq_d, "wq_sb")
            wv_sb = load_w(wv_d, "wv_sb")
            # wo natural [256, 1024] -> 2 k-tiles [128, 1024]
            wo_view = wo_d[:].rearrange("(t p) n -> t p n", p=128)
            wo_sb = []
            for t in range(2):
                w = wopool.tile([128, DIM], F16, name=f"wo{t}", tag="wo")
                nc.sync.dma_start(out=w[:], in_=wo_view[t])
                wo_sb.append(w)

            # V natural [j-tile 128, HL, 65] (col 64 = ones -> denominator)
            v_aug = [vpool.tile([128, HL, HD + 1], F16, name="va", tag="va")
                     for _ in range(JT)]
            for va in v_aug:
                nc.vector.memset(va[:, :, HD:HD + 1], 1.0)

            # ---- projections ----
            # per-quarter tiles keep Tile deps fine-grained
            QT = [[None] * NQ for _ in range(2)]
            KTt = [[None] * NQ for _ in range(2)]

            def qk_proj_quarter(w_sb, tiles, qi, on_act=False):
                for n in range(2):
                    ps = ps_proj.tile([128, QLF], F32, name="ps_p", tag="ps_p")
                    for k in range(KT):
                        nc.tensor.matmul(
                            ps[:],
                            lhsT=w_sb[:, k, n * 128:(n + 1) * 128],
                            rhs=xT_k[qi][k][:],
                            start=(k == 0), stop=(k == KT - 1))
                    t = qkpool.tile([128, QLF], F16, name="qkt", tag="qkt")
                    if on_act:
                        # ACT is idle before the first exp: use it as a second
                        # copy engine so the proj pipeline is PE-bound
                        nc.scalar.activation(
                            out=t[:], in_=ps[:],
                            func=mybir.ActivationFunctionType.Copy)
                    else:
                        nc.vector.tensor_copy(out=t[:], in_=ps[:])
                    tiles[n][qi] = t

            def v_proj_quarter(qi):
                for m in range(QLF // 128):
                    ps = ps_proj.tile([128, DL], F32, name="ps_v", tag="ps_p")
                    for k in range(KT):
                        nc.tensor.matmul(
                            ps[:],
                            lhsT=xT_k[qi][k][:, m * 128:(m + 1) * 128],
                            rhs=wv_sb[:, k, :],
                            start=(k == 0), stop=(k == KT - 1))
                    nc.vector.tensor_copy(
                        out=v_aug[qi * (QLF // 128) + m][:, :, 0:HD],
                        in_=ps[:].rearrange("p (h d) -> p h d", d=HD))

            # K fully first (scores need all j), then Q chunk 0 -> the first
            # scores+exp start ~20us in; V and remaining Q ride behind.
            for qi in range(NQ):
                qk_proj_quarter(wk_sb, KTt, qi, on_act=True)
            qk_proj_quarter(wq_sb, QT, 0, on_act=True)

            # ---- attention ----
            def scores_head(c, h):
                """scores + exp for (chunk c, head h) -> 8 fp16 attn tiles
                [128 j, 2*CH] (j-tile pairs side by side)."""
                ht, hr = h // 2, 64 * (h % 2)
                at_tiles = []
                for jp in range(JT // 2):
                    ps_sc = ps_s.tile([128, 2 * CH], F32, name="ps_sc", tag="ps_s")
                    for s in range(2):
                        j = 2 * jp + s
                        nc.tensor.matmul(
                            ps_sc[:, s * CH:(s + 1) * CH],
                            lhsT=KTt[ht][j // 4][hr:hr + 64,
                                                 (j % 4) * 128:(j % 4 + 1) * 128],
                            rhs=QT[ht][c][hr:hr + 64, :],
                            start=True, stop=True)
                    at = atpool.tile([128, 2 * CH], F16, name="at", tag="at")
                    nc.scalar.activation(
                        out=at[:], in_=ps_sc[:],
                        func=mybir.ActivationFunctionType.Exp,
                        scale=1.0 / np.sqrt(HD).item())
                    at_tiles.append(at)
                return at_tiles

            def av_head(c, h, at_tiles, ao2_tiles):
                """AV natural + per-partition normalize -> writes the head's
                64 columns of the pair tiles ao2 [128 i, 128]."""
                off = (h % 2) * HD
                for ib in range(CH // 128):
                    ps_a = ps_av.tile([128, HD + 1], F32, name="ps_a", tag="ps_av")
                    for j in range(JT):
                        nc.tensor.matmul(
                            ps_a[:],
                            lhsT=at_tiles[j // 2][:, (j % 2) * CH + ib * 128:
                                                  (j % 2) * CH + (ib + 1) * 128],
                            rhs=v_aug[j][:, h, :],
                            start=(j == 0), stop=(j == JT - 1))
                    rec = small.tile([128, 1], F32, name="rec", tag="rec")
                    nc.vector.reciprocal(rec[:], ps_a[:, HD:HD + 1])
                    nc.vector.tensor_scalar_mul(
                        ao2_tiles[h // 2][ib][:, off:off + HD],
                        ps_a[:, 0:HD], rec[:])

            def transpose_pair(ao2, aoT2, p):
                for ib in range(CH // 128):
                    t = aotpool.tile([128, 128], F16, name="aoT2", tag="aoT2")
                    nc.sync.dma_start(out=t[:], in_=ao2[p][ib][:],
                                      transpose=True)
                    aoT2[p][ib] = t

            def wo_ib(c, aoT2_tiles, ib):
                i0 = c * CH
                for half in range(2):
                    ps_o = ps_proj.tile([128, 512], F32, name="ps_o",
                                        tag="ps_p")
                    for p in range(2):
                        nc.tensor.matmul(
                            ps_o[:],
                            lhsT=aoT2_tiles[p][ib][:],
                            rhs=wo_sb[p][:, half * 512:(half + 1) * 512],
                            start=(p == 0), stop=(p == 1))
                    osb = outpool.tile([128, 512], F32, name="osb", tag="osb")
                    nc.vector.tensor_copy(out=osb[:], in_=ps_o[:])
                    nc.sync.dma_start(
                        out=out_d[i0 + ib * 128:i0 + (ib + 1) * 128,
                                  half * 512:(half + 1) * 512],
                        in_=osb[:])

            def new_ao2(c):
                return [[aopool.tile([128, 128], F16, name="ao2", tag="ao2")
                         for _ in range(CH // 128)] for _ in range(2)]

            # chunk 0 scores first (ACT starts early), then V + rest of Q
            at_cur = [scores_head(0, h) for h in range(HL)]
            for qi in range(NQ):
                v_proj_quarter(qi)
            for qi in range(1, NQ):
                qk_proj_quarter(wq_sb, QT, qi)

            for c in range(NCH):
                ao2 = new_ao2(c)
                aoT2 = [[None] * (CH // 128) for _ in range(2)]
                at_next = None
                for h in range(HL):
                    # keep PE fed: next chunk's scores interleave with AV
                    if c + 1 < NCH:
                        if at_next is None:
                            at_next = []
                        at_next.append(scores_head(c + 1, h))
                    av_head(c, h, at_cur[h], ao2)
                    # transpose each head pair as soon as it completes so
                    # only pair 1 sits on the critical tail
                    if h == 1:
                        transpose_pair(ao2, aoT2, 0)
                at_cur = at_next
                transpose_pair(ao2, aoT2, 1)
                for ib in range(CH // 128):
                    wo_ib(c, aoT2, ib)
    nc.compile()
    return nc


_NC_CACHE = None


def _get_nc():
    global _NC_CACHE
    if _NC_CACHE is None:
        _NC_CACHE = build_nc()
    return _NC_CACHE


def kernel(q, Wq, Wk, Wv, Wo, _trace=False, _results=None):
    q = np.asarray(q, np.float32)
    WqT = np.asarray(Wq, np.float32).T.astype(np.float16)
    WkT = np.asarray(Wk, np.float32).T.astype(np.float16)
    WvT = np.asarray(Wv, np.float32).T.astype(np.float16)
    WoT = np.asarray(Wo, np.float32).T.astype(np.float16)

    nc = _get_nc()
    in_maps = []
    for c in range(8):
        b, g = c // 4, c % 4
        hs = slice(DL * g, DL * (g + 1))
        in_maps.append({
            "xT": np.ascontiguousarray(q[b].T.astype(np.float16)),
            "wq": np.ascontiguousarray(WqT[:, hs]),
            "wk": np.ascontiguousarray(WkT[:, hs]),
            "wv": np.ascontiguousarray(WvT[:, hs]),
            "wo": np.ascontiguousarray(WoT[hs, :]),
        })
    res = run_bass_kernel_spmd(
        nc, in_maps, core_ids=list(range(8)), trace=_trace)
    if _results is not None:
        _results.append(res)
    out = np.empty((B, L, DIM), np.float32)
    for b in range(B):
        acc = res.results[4 * b]["out"].astype(np.float32)
        for g in range(1, 4):
            acc = acc + res.results[4 * b + g]["out"]
        out[b] = acc
    return out
